# revision 6
# baseline (speedup 1.0000x reference)
"""BinaryLinear kernel for Trainium2 (8 NeuronCores, SPMD).

Computes  out = sign(x) @ sign(W)^T * alpha  for
x: [8192, 2048] f32, W: [2048, 2048] f32, alpha: [1] f32.

Strategy: data-parallel over the token dim (8 shards of 1024 tokens);
W replicated; fp8(E4M3) +-1 DoubleRow matmuls accumulate exactly in
fp32 PSUM; drains scale by alpha and write fp16 (exact for these small
even-integer outputs).

Input shipping (v3): the op only needs each element's sign, and the
DMA fabric (~280 GB/s aggregate) cannot ship sign BYTES (6 MiB/core)
fast enough to feed the PE's 216ns/matmul cadence, so most data goes
as NIBBLES (2 signs/byte, 3.2 MiB/core):
- byte heads for kt0-1 (x: 256K, W n0: 128K): one DVE op
  ((b&0x80)|0x38) -> fp8 +-1, so the first matmul fires at ~10.5us.
- nibble chunks pack an early k-tile's sign in the HIGH nibble
  (0x3/0xB -> fp8 high nibble of +-1) and a late k-tile's in the LOW:
  pass A ((b&0xF0)|0x08, one full-rate DVE op) yields the high half in
  time for the n=0 k-cadence; pass B (DVE shift/mask, then OR 0x08 on
  the otherwise-idle GpSimd) fills the late half, whose deadlines are
  slack.
Queues: sync x, scalar W, gpsimd alpha + pass-B ORs + output writes
(keeping output descriptor-gen off the drain engine). All drains on
DVE (no ScalarE activations -> no ACT table load delaying DMAs).
n=0 runs k-middle/m-inner; n=1..3 m-outer/k-inner.
"""

import numpy as np

import concourse.bass as bass
import concourse.tile as tile
from concourse import bacc, mybir
from concourse.bass_utils import run_bass_kernel_spmd

N_CORES = 8
NTOK = 8192
INF = 2048
OUTF = 2048
TPC = NTOK // N_CORES  # tokens per core (1024)
P = 128
KT = INF // P  # 16 contraction tiles
MT = TPC // P  # 8 token tiles per core
NTS = 512  # out_features per matmul (one PSUM bank)
NT = OUTF // NTS  # 4

F32 = mybir.dt.float32
F16 = mybir.dt.float16
FP8 = mybir.dt.float8e4  # E4M3; +-1.0 is exact
U8 = mybir.dt.uint8
U32 = mybir.dt.uint32

N_DUMMY_MM = 24  # warm-up matmuls bridge branch-entry (~7.2us) to ~10.4us

_compiled = None
LAST_RESULT = None  # BassKernelResults of the most recent run (for profiling)


def _build():
    nc = bacc.Bacc(
        "TRN2",
        target_bir_lowering=False,
        debug=False,
        num_devices=N_CORES,
    )
    xh = nc.dram_tensor("xh", [P, 2 * TPC], U8, kind="ExternalInput").ap()
    xnA = nc.dram_tensor("xnA", [P, 4 * TPC], U8, kind="ExternalInput").ap()
    xnB = nc.dram_tensor("xnB", [P, 3 * TPC], U8, kind="ExternalInput").ap()
    wh = nc.dram_tensor("wh", [P, 2 * NTS], U8, kind="ExternalInput").ap()
    w0n = nc.dram_tensor("w0n", [P, 7 * NTS], U8, kind="ExternalInput").ap()
    wn1 = nc.dram_tensor("wn1", [P, 8 * NTS], U8, kind="ExternalInput").ap()
    wn2 = nc.dram_tensor("wn2", [P, 8 * NTS], U8, kind="ExternalInput").ap()
    wn3 = nc.dram_tensor("wn3", [P, 8 * NTS], U8, kind="ExternalInput").ap()
    al = nc.dram_tensor("alpha", [P, 1], F32, kind="ExternalInput").ap()
    out = nc.dram_tensor(
        "out", [NT, MT // 2, P, 2 * NTS], F16, kind="ExternalOutput"
    ).ap()

    with tile.TileContext(nc) as tc:
        with (
            tc.tile_pool(name="res", bufs=1) as res,
            tc.tile_pool(name="tmp", bufs=2) as tmpp,
            tc.tile_pool(name="psum", bufs=8, space="PSUM") as ppool,
            tc.tile_pool(name="outp", bufs=2) as outp,
        ):
            bx = res.tile([P, KT, TPC], FP8, name="bx")
            bw = res.tile([P, KT, NT, NTS], FP8, name="bw")
            alpha_t = res.tile([P, 1], F32)

            xh_r = res.tile([P, 2 * TPC], U8, name="xh_r")
            xnA_r = res.tile([P, 4 * TPC], U8, name="xnA_r")
            xnB_r = res.tile([P, 3 * TPC], U8, name="xnB_r")
            wh_r = res.tile([P, 2 * NTS], U8, name="wh_r")
            w0n_r = res.tile([P, 7 * NTS], U8, name="w0n_r")
            wn_r = [res.tile([P, 8 * NTS], U8, name=f"wn{n}_r")
                    for n in (1, 2, 3)]

            AND, OR = mybir.AluOpType.bitwise_and, mybir.AluOpType.bitwise_or
            SHL = mybir.AluOpType.logical_shift_left

            def u32(ap):
                return ap.bitcast(U32)

            def sign_b(dst, src):  # MSB byte -> fp8 +-1
                nc.vector.tensor_scalar(
                    u32(dst), u32(src), 0x80808080, 0x38383838,
                    op0=AND, op1=OR,
                )

            def passA(dst, src):  # high nibble -> fp8 +-1 (DVE, 1 op)
                nc.vector.tensor_scalar(
                    u32(dst), u32(src), 0xF0F0F0F0, 0x08080808,
                    op0=AND, op1=OR,
                )

            def passB1(tmp, src):  # low nibble <<4, masked (DVE)
                nc.vector.tensor_scalar(
                    u32(tmp), u32(src), 4, 0xF0F0F0F0, op0=SHL, op1=AND
                )

            def passB2(dst, tmp):  # | 0x08 -> fp8 +-1
                nc.vector.tensor_scalar(
                    u32(dst), u32(tmp), 0x08080808, None, op0=OR
                )

            # Warm-up matmuls keep the PE HAM activity monitor busy
            # through the DMA fill so real matmuls run at 2.4GHz.
            dummy = res.tile([P, 2, P], FP8)
            psd = ppool.tile([P, NTS], F32, name="ps", tag="ps")
            nc.gpsimd.memset(dummy[:], 0)
            for _ in range(N_DUMMY_MM):
                nc.tensor.matmul(
                    psd[:, 0:P], dummy[:], dummy[:],
                    start=True, stop=True,
                    perf_mode=mybir.MatmulPerfMode.DoubleRow,
                )

            # ---- load phase ----
            nc.sync.dma_start(xh_r[:], xh)
            nc.scalar.dma_start(wh_r[:], wh)
            nc.gpsimd.dma_start(alpha_t[:], al)
            nc.sync.dma_start(xnA_r[:], xnA)
            nc.scalar.dma_start(w0n_r[:], w0n)
            nc.sync.dma_start(xnB_r[:], xnB)
            nc.scalar.dma_start(wn_r[0][:], wn1)
            nc.scalar.dma_start(wn_r[1][:], wn2)
            nc.scalar.dma_start(wn_r[2][:], wn3)

            # ---- expansion (DVE in deadline order; ORs on GpSimd) ----
            def rs(raw, a, b):
                # contiguous [P, a*b] raw buffer viewed as [P, a, b]
                return raw.rearrange("p (a b) -> p a b", a=a)

            def b_chain(dst, raw, a, b):
                t = tmpp.tile([P, 4 * TPC], U8, name="t", tag="t")
                tv = rs(t[:, 0 : a * b], a, b)
                passB1(tv, rs(raw, a, b))
                passB2(dst, tv)

            sign_b(bw[:, 0:2, 0, :], rs(wh_r[:], 2, NTS))  # W kt0-1 n0
            sign_b(bx[:, 0:2, :], rs(xh_r[:], 2, TPC))     # x kt0-1
            passA(bw[:, 2:9, 0, :], rs(w0n_r[:], 7, NTS))  # W kt2-8 n0
            passA(bx[:, 2:6, :], rs(xnA_r[:], 4, TPC))     # x kt2-5
            passA(bx[:, 6:9, :], rs(xnB_r[:], 3, TPC))     # x kt6-8
            b_chain(bx[:, 9:13, :], xnA_r[:], 4, TPC)      # x kt9-12
            b_chain(bw[:, 9:16, 0, :], w0n_r[:], 7, NTS)   # W kt9-15 n0
            passA(bw[:, 0:8, 1, :], rs(wn_r[0][:], 8, NTS))  # W kt0-7 n1
            b_chain(bx[:, 13:16, :], xnB_r[:], 3, TPC)     # x kt13-15
            b_chain(bw[:, 8:16, 1, :], wn_r[0][:], 8, NTS)
            passA(bw[:, 0:8, 2, :], rs(wn_r[1][:], 8, NTS))
            b_chain(bw[:, 8:16, 2, :], wn_r[1][:], 8, NTS)
            passA(bw[:, 0:8, 3, :], rs(wn_r[2][:], 8, NTS))
            b_chain(bw[:, 8:16, 3, :], wn_r[2][:], 8, NTS)

            def mm(ps_ap, m, n, k):
                nc.tensor.matmul(
                    ps_ap,
                    bx[:, k : k + 2, m * P : (m + 1) * P],
                    bw[:, k : k + 2, n, :],
                    start=(k == 0),
                    stop=(k + 2 >= KT),
                    perf_mode=mybir.MatmulPerfMode.DoubleRow,
                )

            def drain(dst, ps):
                nc.vector.tensor_scalar_mul(dst, ps, alpha_t[:])

            def store_pair(obuf, n, m):
                nc.gpsimd.dma_start(
                    out[n, m // 2],
                    obuf[:, m - 1 : m + 1, :].rearrange("p a b -> p (a b)"),
                )

            # ---- matmul phase ----
            # n=0: k-middle / m-inner so matmuls start on the first k-pair.
            obuf = outp.tile([P, MT, NTS], F16)
            pss = [
                ppool.tile([P, NTS], F32, name="ps", tag="ps")
                for _ in range(MT)
            ]
            for k in range(0, KT, 2):
                for m in range(MT):
                    mm(pss[m][:], m, 0, k)
            for m in range(MT):
                drain(obuf[:, m, :], pss[m][:])
                if m % 2 == 1:
                    store_pair(obuf, 0, m)

            # n=1..3: m-outer / k-inner; drain overlaps the next m's MMs.
            for n in range(1, NT):
                obuf = outp.tile([P, MT, NTS], F16)
                for m in range(MT):
                    ps = ppool.tile([P, NTS], F32, name="ps", tag="ps")
                    for k in range(0, KT, 2):
                        mm(ps[:], m, n, k)
                    drain(obuf[:, m, :], ps[:])
                    if m % 2 == 1:
                        store_pair(obuf, n, m)

    nc.compile()
    return nc


def _msb(a):
    # MSB byte of each little-endian f32: sign bit + top exponent bits.
    return a.view(np.uint8).reshape(a.shape[0], a.shape[1], 4)[:, :, 3]


def _nib(hi_sign, lo_sign):
    # sign bits (bool arrays) -> packed nibble bytes: fp8 +-1 high nibble
    # (0x3/0xB) in the byte's high nibble, low k-tile's in the low.
    return (
        np.where(hi_sign, 0xB0, 0x30) | np.where(lo_sign, 0x0B, 0x03)
    ).astype(np.uint8)


def _pack_w(weight):
    w4 = _msb(weight).T.reshape(KT, P, NT, NTS)  # [kt, p, n, c]
    s = w4 >= 0x80
    wh = np.ascontiguousarray(
        w4[0:2, :, 0, :].transpose(1, 0, 2).reshape(P, 2 * NTS)
    )
    w0n = np.ascontiguousarray(
        _nib(s[2:9, :, 0, :], s[9:16, :, 0, :])
        .transpose(1, 0, 2).reshape(P, 7 * NTS)
    )
    wns = [
        np.ascontiguousarray(
            _nib(s[0:8, :, n, :], s[8:16, :, n, :])
            .transpose(1, 0, 2).reshape(P, 8 * NTS)
        )
        for n in (1, 2, 3)
    ]
    return wh, w0n, wns


def _pack_x_shard(xs):
    x4 = _msb(xs).T.reshape(KT, P, TPC)  # [kt, p, t]
    s = x4 >= 0x80
    xh = np.ascontiguousarray(
        x4[0:2].transpose(1, 0, 2).reshape(P, 2 * TPC)
    )
    xnA = np.ascontiguousarray(
        _nib(s[2:6], s[9:13]).transpose(1, 0, 2).reshape(P, 4 * TPC)
    )
    xnB = np.ascontiguousarray(
        _nib(s[6:9], s[13:16]).transpose(1, 0, 2).reshape(P, 3 * TPC)
    )
    return xh, xnA, xnB


def kernel(x, weight, alpha):
    global _compiled, LAST_RESULT
    if _compiled is None:
        _compiled = _build()
    nc = _compiled

    x = np.asarray(x, dtype=np.float32)
    weight = np.asarray(weight, dtype=np.float32)
    alpha = np.asarray(alpha, dtype=np.float32)

    wh, w0n, wns = _pack_w(weight)
    alv = np.full((P, 1), alpha.reshape(-1)[0], dtype=np.float32)
    in_maps = []
    for c in range(N_CORES):
        xh, xnA, xnB = _pack_x_shard(x[c * TPC : (c + 1) * TPC, :])
        in_maps.append({
            "xh": xh, "xnA": xnA, "xnB": xnB,
            "wh": wh, "w0n": w0n,
            "wn1": wns[0], "wn2": wns[1], "wn3": wns[2],
            "alpha": alv,
        })

    LAST_RESULT = run_bass_kernel_spmd(nc, in_maps, list(range(N_CORES)))
    outs = []
    for c in range(N_CORES):
        o = LAST_RESULT.results[c]["out"]  # [NT, MT//2, P, 2*NTS] f16
        o = o.reshape(NT, MT // 2, P, 2, NTS).astype(np.float32)
        outs.append(o.transpose(1, 3, 2, 0, 4).reshape(TPC, OUTF))
    return np.concatenate(outs, axis=0)


# revision 9
# speedup vs baseline: 1.1051x; 1.1051x over previous
"""BinaryLinear kernel for Trainium2 (8 NeuronCores, SPMD).

Computes  out = sign(x) @ sign(W)^T * alpha  for
x: [8192, 2048] f32, W: [2048, 2048] f32, alpha: [1] f32.

Strategy: data-parallel over the token dim (8 shards of 1024 tokens);
W replicated; fp8(E4M3) +-1 DoubleRow matmuls accumulate exactly in
fp32 PSUM; drains scale by alpha and write fp16 (exact for these small
even-integer outputs).

Input shipping (v4): the op only needs each element's sign, and the
DMA fabric (~280 GB/s aggregate) cannot ship sign BYTES (6 MiB/core)
fast enough to feed the PE's 216 ns/matmul cadence. So signs travel as
NIBBLES (2 signs/byte, 1.66 MiB/core): a byte packs an early k-tile's
sign in its HIGH nibble (0x3/0xB = the high nibble of fp8 +-1) and a
late k-tile's in the LOW nibble. Pass A ((b & 0xF0) | 0x08), a single
full-rate DVE op, yields the early k-tile in time for the n=0
k-cadence; pass B (shift/mask, then OR 0x08) fills the late k-tile,
whose deadline is ~7us slack. Expansion targets are per-chunk
CONTIGUOUS regions (per-n W tiles, k-contiguous chunks) so the tile
framework's interval-based overlap tracking derives true minimal
dependencies (a shared 4D tile's interleaved writes serialize every
matmul behind the whole expansion).

Queues: x nibbles alternate sync/gpsimd (x needs ~150 GB/s sustained,
more than one queue's fair share); W nibbles on scalar; alpha + output
writes on gpsimd (descriptor-gen never delays a drain). All drains on
DVE; no ScalarE activations -> no ACT table load ahead of the first
DMAs. n=0 runs k-middle/m-inner so the first matmul needs only one
x and one W chunk; n=1..3 run m-outer/k-inner.
"""

import numpy as np

import concourse.bass as bass
import concourse.tile as tile
from concourse import bacc, mybir
from concourse.bass_utils import run_bass_kernel_spmd

N_CORES = 8
NTOK = 8192
INF = 2048
OUTF = 2048
TPC = NTOK // N_CORES  # tokens per core (1024)
P = 128
KT = INF // P  # 16 contraction tiles
MT = TPC // P  # 8 token tiles per core
NTS = 512  # out_features per matmul (one PSUM bank)
NT = OUTF // NTS  # 4

F32 = mybir.dt.float32
F16 = mybir.dt.float16
FP8 = mybir.dt.float8e4  # E4M3; +-1.0 is exact
U8 = mybir.dt.uint8
U32 = mybir.dt.uint32

N_DUMMY_MM = 24  # warm-up matmuls bridge branch-entry (~7.2us) to ~10.4us

_compiled = None
LAST_RESULT = None  # BassKernelResults of the most recent run (for profiling)


def _build():
    nc = bacc.Bacc(
        "TRN2",
        target_bir_lowering=False,
        debug=False,
        num_devices=N_CORES,
    )
    xns = [
        nc.dram_tensor(f"xn{c}", [P, 2 * TPC], U8, kind="ExternalInput").ap()
        for c in range(4)
    ]
    w0s = [
        nc.dram_tensor(f"w0c{c}", [P, 2 * NTS], U8, kind="ExternalInput").ap()
        for c in range(4)
    ]
    wns = [
        nc.dram_tensor(f"wn{n}", [P, 8 * NTS], U8, kind="ExternalInput").ap()
        for n in (1, 2, 3)
    ]
    al = nc.dram_tensor("alpha", [P, 1], F32, kind="ExternalInput").ap()
    out = nc.dram_tensor(
        "out", [NT, MT // 2, P, 2 * NTS], F16, kind="ExternalOutput"
    ).ap()

    with tile.TileContext(nc) as tc:
        with (
            tc.tile_pool(name="res", bufs=1) as res,
            tc.tile_pool(name="tmp", bufs=2) as tmpp,
            tc.tile_pool(name="psum", bufs=8, space="PSUM") as ppool,
            tc.tile_pool(name="outp", bufs=2) as outp,
        ):
            bx = res.tile([P, KT, TPC], FP8, name="bx")
            bws = [res.tile([P, KT, NTS], FP8, name=f"bw{n}")
                   for n in range(NT)]
            alpha_t = res.tile([P, 1], F32)

            xn_r = [res.tile([P, 2 * TPC], U8, name=f"xn{c}_r")
                    for c in range(4)]
            w0_r = [res.tile([P, 2 * NTS], U8, name=f"w0c{c}_r")
                    for c in range(4)]
            wn_r = [res.tile([P, 8 * NTS], U8, name=f"wn{n}_r")
                    for n in (1, 2, 3)]

            AND, OR = mybir.AluOpType.bitwise_and, mybir.AluOpType.bitwise_or
            SHL = mybir.AluOpType.logical_shift_left

            def u32(ap):
                return ap.bitcast(U32)

            def passA(dst, src):  # high nibble -> fp8 +-1 (1 DVE op)
                nc.vector.tensor_scalar(
                    u32(dst), u32(src), 0xF0F0F0F0, 0x08080808,
                    op0=AND, op1=OR,
                )

            def rs(raw, a, b):
                return raw.rearrange("p (a b) -> p a b", a=a)

            def b_chain(dst, raw, a, b):  # low nibble -> fp8 +-1 (2 ops)
                t = tmpp.tile([P, 8 * NTS], U8, name="t", tag="t")
                tv = rs(t[:, 0 : a * b], a, b)
                nc.vector.tensor_scalar(
                    u32(tv), u32(rs(raw, a, b)), 4, 0xF0F0F0F0,
                    op0=SHL, op1=AND,
                )
                nc.vector.tensor_scalar(
                    u32(dst), u32(tv), 0x08080808, None, op0=OR
                )

            # Warm-up matmuls keep the PE HAM activity monitor busy
            # through the DMA fill so real matmuls run at 2.4GHz.
            dummy = res.tile([P, 2, P], FP8)
            psd = ppool.tile([P, NTS], F32, name="ps", tag="ps")
            nc.gpsimd.memset(dummy[:], 0)
            for _ in range(N_DUMMY_MM):
                nc.tensor.matmul(
                    psd[:, 0:P], dummy[:], dummy[:],
                    start=True, stop=True,
                    perf_mode=mybir.MatmulPerfMode.DoubleRow,
                )

            # ---- load phase ----
            nc.sync.dma_start(xn_r[0][:], xns[0])
            nc.gpsimd.dma_start(alpha_t[:], al)
            nc.gpsimd.dma_start(xn_r[1][:], xns[1])
            nc.scalar.dma_start(w0_r[0][:], w0s[0])
            nc.sync.dma_start(xn_r[2][:], xns[2])
            nc.gpsimd.dma_start(xn_r[3][:], xns[3])
            nc.scalar.dma_start(w0_r[1][:], w0s[1])
            nc.scalar.dma_start(w0_r[2][:], w0s[2])
            nc.scalar.dma_start(w0_r[3][:], w0s[3])
            nc.scalar.dma_start(wn_r[0][:], wns[0])
            nc.scalar.dma_start(wn_r[1][:], wns[1])
            nc.scalar.dma_start(wn_r[2][:], wns[2])

            # ---- expansion on DVE, in deadline order ----
            # chunk c: high nibbles -> kt 2c..2c+1, low -> kt 8+2c..9+2c
            passA(bws[0][:, 0:2, :], rs(w0_r[0][:], 2, NTS))
            passA(bx[:, 0:2, :], rs(xn_r[0][:], 2, TPC))
            passA(bws[0][:, 2:4, :], rs(w0_r[1][:], 2, NTS))
            passA(bx[:, 2:4, :], rs(xn_r[1][:], 2, TPC))
            passA(bws[0][:, 4:6, :], rs(w0_r[2][:], 2, NTS))
            passA(bx[:, 4:6, :], rs(xn_r[2][:], 2, TPC))
            passA(bws[0][:, 6:8, :], rs(w0_r[3][:], 2, NTS))
            passA(bx[:, 6:8, :], rs(xn_r[3][:], 2, TPC))
            b_chain(bws[0][:, 8:10, :], w0_r[0][:], 2, NTS)
            b_chain(bx[:, 8:10, :], xn_r[0][:], 2, TPC)
            b_chain(bws[0][:, 10:12, :], w0_r[1][:], 2, NTS)
            b_chain(bx[:, 10:12, :], xn_r[1][:], 2, TPC)
            b_chain(bws[0][:, 12:14, :], w0_r[2][:], 2, NTS)
            b_chain(bx[:, 12:14, :], xn_r[2][:], 2, TPC)
            b_chain(bws[0][:, 14:16, :], w0_r[3][:], 2, NTS)
            b_chain(bx[:, 14:16, :], xn_r[3][:], 2, TPC)
            passA(bws[1][:, 0:8, :], rs(wn_r[0][:], 8, NTS))
            b_chain(bws[1][:, 8:16, :], wn_r[0][:], 8, NTS)
            passA(bws[2][:, 0:8, :], rs(wn_r[1][:], 8, NTS))
            b_chain(bws[2][:, 8:16, :], wn_r[1][:], 8, NTS)
            passA(bws[3][:, 0:8, :], rs(wn_r[2][:], 8, NTS))
            b_chain(bws[3][:, 8:16, :], wn_r[2][:], 8, NTS)

            def mm(ps_ap, m, n, k):
                nc.tensor.matmul(
                    ps_ap,
                    bx[:, k : k + 2, m * P : (m + 1) * P],
                    bws[n][:, k : k + 2, :],
                    start=(k == 0),
                    stop=(k + 2 >= KT),
                    perf_mode=mybir.MatmulPerfMode.DoubleRow,
                )

            def drain(dst, ps):
                nc.vector.tensor_scalar_mul(dst, ps, alpha_t[:])

            def store_pair(obuf, n, m):
                nc.gpsimd.dma_start(
                    out[n, m // 2],
                    obuf[:, m - 1 : m + 1, :].rearrange("p a b -> p (a b)"),
                )

            # ---- matmul phase ----
            # n=0: k-middle / m-inner so matmuls start on the first k-pair.
            obuf = outp.tile([P, MT, NTS], F16)
            pss = [
                ppool.tile([P, NTS], F32, name="ps", tag="ps")
                for _ in range(MT)
            ]
            for k in range(0, KT, 2):
                for m in range(MT):
                    mm(pss[m][:], m, 0, k)
            for m in range(MT):
                drain(obuf[:, m, :], pss[m][:])
                if m % 2 == 1:
                    store_pair(obuf, 0, m)

            # n=1..3: m-outer / k-inner; drain overlaps the next m's MMs.
            for n in range(1, NT):
                obuf = outp.tile([P, MT, NTS], F16)
                for m in range(MT):
                    ps = ppool.tile([P, NTS], F32, name="ps", tag="ps")
                    for k in range(0, KT, 2):
                        mm(ps[:], m, n, k)
                    drain(obuf[:, m, :], ps[:])
                    if m % 2 == 1:
                        store_pair(obuf, n, m)

    nc.compile()
    return nc


def _msb(a):
    # MSB byte of each little-endian f32: sign bit + top exponent bits.
    return a.view(np.uint8).reshape(a.shape[0], a.shape[1], 4)[:, :, 3]


def _nib(hi_sign, lo_sign):
    # sign bits -> packed nibble bytes: fp8 +-1's high nibble (0x3/0xB)
    # for the early k-tile in the byte's high nibble, late in the low.
    return (
        np.where(hi_sign, 0xB0, 0x30) | np.where(lo_sign, 0x0B, 0x03)
    ).astype(np.uint8)


def _pack_w(weight):
    w4 = _msb(weight).T.reshape(KT, P, NT, NTS)  # [kt, p, n, c]
    s = w4 >= 0x80
    w0s = [
        np.ascontiguousarray(
            _nib(s[2 * c : 2 * c + 2, :, 0, :],
                 s[8 + 2 * c : 10 + 2 * c, :, 0, :])
            .transpose(1, 0, 2).reshape(P, 2 * NTS)
        )
        for c in range(4)
    ]
    wns = [
        np.ascontiguousarray(
            _nib(s[0:8, :, n, :], s[8:16, :, n, :])
            .transpose(1, 0, 2).reshape(P, 8 * NTS)
        )
        for n in (1, 2, 3)
    ]
    return w0s, wns


def _pack_x_shard(xs):
    x4 = _msb(xs).T.reshape(KT, P, TPC)  # [kt, p, t]
    s = x4 >= 0x80
    return [
        np.ascontiguousarray(
            _nib(s[2 * c : 2 * c + 2], s[8 + 2 * c : 10 + 2 * c])
            .transpose(1, 0, 2).reshape(P, 2 * TPC)
        )
        for c in range(4)
    ]


def kernel(x, weight, alpha):
    global _compiled, LAST_RESULT
    if _compiled is None:
        _compiled = _build()
    nc = _compiled

    x = np.asarray(x, dtype=np.float32)
    weight = np.asarray(weight, dtype=np.float32)
    alpha = np.asarray(alpha, dtype=np.float32)

    w0s, wns = _pack_w(weight)
    alv = np.full((P, 1), alpha.reshape(-1)[0], dtype=np.float32)
    in_maps = []
    for c in range(N_CORES):
        xcs = _pack_x_shard(x[c * TPC : (c + 1) * TPC, :])
        m = {f"xn{i}": xcs[i] for i in range(4)}
        m.update({f"w0c{i}": w0s[i] for i in range(4)})
        m.update({f"wn{n}": wns[n - 1] for n in (1, 2, 3)})
        m["alpha"] = alv
        in_maps.append(m)

    LAST_RESULT = run_bass_kernel_spmd(nc, in_maps, list(range(N_CORES)))
    outs = []
    for c in range(N_CORES):
        o = LAST_RESULT.results[c]["out"]  # [NT, MT//2, P, 2*NTS] f16
        o = o.reshape(NT, MT // 2, P, 2, NTS).astype(np.float32)
        outs.append(o.transpose(1, 3, 2, 0, 4).reshape(TPC, OUTF))
    return np.concatenate(outs, axis=0)
